# revision 19
# baseline (speedup 1.0000x reference)
# Depthwise causal conv2d (N=2, C=16, H=W=2048, kernel 6x11) on 8 TRN2 cores.
#
# y[b,c,p,q] = sum_{r,s} w[c,r,s] * xm[b,c, p+r-5, q+s-5], xm = tril-masked x,
# y tril-masked.  Sharding: the 32 (b,c) images are independent; 4 per core.
#
# Column-banded transposed formulation: both x and y live TRANSPOSED in DRAM
# (xT[u=w, v=h], yT[q, p]; host transposes outside the kernel launch).  Then
#   yT[q, p] = sum_r sum_k B_r[k, q-q0] * xT[q0-5+k, p+r-5]
# with B_r[k, m] = w[c, r, k-m] (band-11 Toeplitz over the S taps).  Per
# output tile [QM=118 q x 512 p] that is SIX accumulating matmuls (one per
# row tap r; stationary = B_r, moving = a column-shifted slice of the xT
# strip) instead of the eleven the row-banded form needs: the 11-wide S band
# packs the contraction denser than the 6-wide R band.
#
# DMA: one load per (image, q0-strip) [128, <=2053] and one store per strip
# (outputs staged in SBUF as bf16), issued alternately on the SP and
# Activation HWDGE rings - ~2.5x fewer descriptors than per-tile DMA and two
# sequencers instead of one.  yT is written bf16 (halves store bytes); the
# host widens to fp32.  Causal masks: gpsimd affine_select on the input
# strip (keep v >= u), DVE staircase multiply on PSUM evacuation.
import sys

sys.path.insert(0, "/opt/trn_rl_repo")

import numpy as np

import concourse.bacc as bacc
import concourse.mybir as mybir
import concourse.tile as tile
from concourse.bass_utils import run_bass_kernel_spmd

N, C, H, W = 2, 16, 2048, 2048
R, S, PH, PW = 6, 11, 5, 5
NCORES = 8
IPC = (N * C) // NCORES  # images per core
QM = 118      # output cols (q) per strip: 128-partition contraction window
PN = 512      # output rows (p) per tile (one PSUM bank of fp32)
STW = 1152    # staircase width; stair2[i, t] = 1 iff t >= i + 512
F32 = mybir.dt.float32
BF16 = mybir.dt.bfloat16

_NC_CACHE = {}


def _np_bf16():
    import ml_dtypes

    return np.dtype(ml_dtypes.bfloat16)


def _strips():
    """(q0, qm) col-strips covering q in [0, H)."""
    out = []
    q0 = 0
    while q0 < H:
        out.append((q0, min(QM, H - q0)))
        q0 += QM
    return out


def _p_tiles(q0):
    """Row tiles (p0) with any causal output (p >= q0 somewhere)."""
    return [p0 for p0 in range(0, H, PN) if p0 + PN - 1 >= q0]


def _build_program(rep=1):
    """One SPMD program: conv of IPC transposed images with per-image bands.

    rep > 1 wraps the body in a hardware loop (benchmarking only)."""
    import contextlib

    nc = bacc.Bacc("TRN2", target_bir_lowering=False, debug=False,
                   num_devices=NCORES)
    xT = nc.dram_tensor("xT", [IPC, W, H], BF16, kind="ExternalInput")
    bands = nc.dram_tensor("bands", [IPC, 128, R * 128], BF16,
                           kind="ExternalInput")
    yT = nc.dram_tensor("yT", [IPC, W, H], BF16, kind="ExternalOutput")

    with tile.TileContext(nc) as tc:
        with (
            tc.tile_pool(name="const", bufs=1) as cpool,
            tc.tile_pool(name="xin", bufs=4) as xpool,
            tc.tile_pool(name="out", bufs=4) as opool,
            tc.tile_pool(name="psum", bufs=8, space="PSUM") as ppool,
            tc.For_i(0, rep, 1) if rep > 1 else contextlib.nullcontext(),
        ):
            # Staircase mask: stair2[i, t] = 1 iff t - i - 512 >= 0.
            stair2 = cpool.tile([128, STW], F32)
            nc.gpsimd.memset(stair2[:], 1.0)
            nc.gpsimd.affine_select(
                out=stair2[:], in_=stair2[:],
                compare_op=mybir.AluOpType.is_ge, fill=0.0,
                base=-512, channel_multiplier=-1,
                pattern=[[1, STW]],
            )

            ndma = [0]
            issuers = [nc.sync, nc.scalar, nc.gpsimd]

            def dma(out, in_, nsplit=1):
                # Split along the partition dim (32-row chunks spread across
                # the SDMA engines; bigger chunks funnel onto one) and rotate
                # the issuing sequencer (2 HWDGE rings + SWDGE).
                np_ = out.shape[0]
                assert in_.shape[0] == np_
                step = (np_ + nsplit - 1) // nsplit
                for a in range(0, np_, step):
                    b = min(np_, a + step)
                    eng = issuers[ndma[0] % len(issuers)]
                    ndma[0] += 1
                    eng.dma_start(out=out[a:b], in_=in_[a:b])

            bt = cpool.tile([128, IPC * R * 128], BF16)
            for i in range(IPC):
                # Band load emitted at image start: overlaps the previous
                # image's tail instead of serializing the kernel prologue.
                band_i = bt[:, i * R * 128:(i + 1) * R * 128]
                dma(band_i, bands[i], nsplit=4)
                for (q0, qm) in _strips():
                    _emit_strip(nc, tc, xpool, opool, ppool, xT, yT,
                                band_i, stair2, dma, i, q0, qm)
    nc.compile()
    return nc


def _emit_strip(nc, tc, xpool, opool, ppool, xT, yT, band_i, stair2, dma,
                i, q0, qm):
    p_tiles = _p_tiles(q0)
    ps0 = p_tiles[0]
    v0 = ps0 - PH            # strip col range [v0, H) in v = p coords
    ext = H - v0             # SBUF strip width (incl. 5-col lead halo)
    u0 = q0 - PH             # partition k = u - u0, u = input col
    uv0, uv1 = max(0, u0), min(W, u0 + 128)  # valid u rows to load

    xt = xpool.tile([128, ext], BF16, tag="xin")
    # x is causally pre-masked on the host (xT[u, v] = 0 for v < u), so no
    # in-kernel selects: just zero the regions the DMA does not write.
    # Cols v < u0 are entirely below every u in this strip's window (all
    # masked zeros): memset instead of loading.
    lv0 = max(0, v0, u0)
    if u0 + 128 > W:
        # last strip: partitions k >= W-u0 are u >= W (right w-padding).
        # Compute-engine partition bases must be 32-aligned, so zero the
        # whole tile first; the load below overwrites the valid rows (WAW).
        nc.gpsimd.memset(xt[:, :], 0.0)
    dma(xt[uv0 - u0:uv1 - u0, lv0 - v0:], xT[i, uv0:uv1, lv0:H], nsplit=4)
    if lv0 > v0:
        # covers both the below-diagonal clip and the v<0 top padding
        nc.gpsimd.memset(xt[:, :lv0 - v0], 0.0)
    if u0 < 0:
        # q0 == 0: partitions k < 5 are u < 0 (left w-padding): zero them.
        nc.gpsimd.memset(xt[:PH, :], 0.0)

    ys = opool.tile([128, ext], BF16, tag="out")
    # r-outer / p-inner: consecutive matmuls share the stationary band B_r;
    # the explicit ldweights lets codegen elide the per-matmul weight load.
    pts = {}
    for p0 in p_tiles:
        pt = ppool.tile([128, PN], F32, tag="psum")
        pts[p0] = pt
    for r in range(R):
        band_r = band_i[:, r * 128:(r + 1) * 128]
        for p0 in p_tiles:
            # moving operand: xT strip cols v = p0-5+r .. +PN.  Cols left
            # of the causal diagonal (n < n0) are masked on evacuation:
            # skip streaming them.
            n0 = max(0, q0 - p0)
            j0 = p0 - PH + r - v0
            nc.tensor.matmul(
                pts[p0][:, n0:PN],
                lhsT=band_r,
                rhs=xt[:, j0 + n0:j0 + PN],
                start=(r == 0), stop=(r == R - 1),
            )
    for p0 in p_tiles:
        pt = pts[p0]
        # Evacuate cols p >= q0 only (left of that is all-masked).
        n0 = max(0, q0 - p0)
        d = p0 - q0
        crossing = p0 + n0 < q0 + qm - 1
        if crossing:
            nc.vector.tensor_mul(
                ys[:qm, p0 + n0 - v0:p0 + PN - v0],
                pt[:qm, n0:PN],
                stair2[:qm, 512 + d + n0:512 + d + PN],
            )
        else:
            nc.vector.tensor_copy(
                ys[:qm, p0 + n0 - v0:p0 + PN - v0], pt[:qm, n0:PN],
            )
    # One store per strip: cols p in [q0, H).
    dma(yT[i, q0:q0 + qm, q0:H], ys[:qm, q0 - v0:], nsplit=4)


def _build_bands(weight):
    """Host-side: per-image column-banded Toeplitz weights.
    bands[img, k, r*128 + m] = w[c(img), r, k-m] for k-m in [0, S)."""
    nimg = N * C
    bands = np.zeros((nimg, 128, R * 128), np.float32)
    m = np.arange(128)
    for r in range(R):
        for s in range(S):
            valid = m + s < 128
            mv = m[valid]
            for img in range(nimg):
                c = img % C
                bands[img, mv + s, r * 128 + mv] = weight[c, r, s]
    return bands.astype(_np_bf16())


def _prep_xT(x):
    """Causally masked (h >= w), transposed, bf16 images [N*C, W, H]."""
    mask = np.tril(np.ones((H, W), np.float32))
    xm = x.reshape(N * C, H, W) * mask
    return np.ascontiguousarray(xm.swapaxes(1, 2)).astype(
        _np_bf16(), copy=False)


def kernel(x, weight):
    x = np.asarray(x, dtype=np.float32)
    weight = np.asarray(weight, dtype=np.float32)
    assert x.shape == (N, C, H, W) and weight.shape == (C, R, S)

    if "nc" not in _NC_CACHE:
        _NC_CACHE["nc"] = _build_program()
    nc = _NC_CACHE["nc"]

    xT_imgs = _prep_xT(x)
    bands = _build_bands(weight)
    in_maps = [
        {
            "xT": xT_imgs[k * IPC:(k + 1) * IPC],
            "bands": bands[k * IPC:(k + 1) * IPC],
        }
        for k in range(NCORES)
    ]
    res = run_bass_kernel_spmd(nc, in_maps, list(range(NCORES)))
    yT = np.concatenate([res.results[k]["yT"] for k in range(NCORES)], axis=0)
    y = yT.astype(np.float32).swapaxes(1, 2)
    return np.ascontiguousarray(y).reshape(N, C, H, W)


# revision 22
# speedup vs baseline: 1.7347x; 1.7347x over previous
# Depthwise causal conv2d (N=2, C=16, H=W=2048, kernel 6x11) on 8 TRN2 cores.
#
# y[b,c,p,q] = sum_{r,s} w[c,r,s] * xm[b,c, p+r-5, q+s-5], xm = tril-masked x,
# y tril-masked.  Sharding: the 32 (b,c) images are independent; 4 per core.
#
# Column-banded transposed formulation: both x and y live TRANSPOSED in DRAM
# (xT[u=w, v=h], yT[q, p]; host transposes outside the kernel launch).  Then
#   yT[q, p] = sum_r sum_k B_r[k, q-q0] * xT[q0-5+k, p+r-5]
# with B_r[k, m] = w[c, r, k-m] (band-11 Toeplitz over the S taps).  Per
# output tile [QM=118 q x 512 p] that is SIX accumulating matmuls (one per
# row tap r; stationary = B_r, moving = a column-shifted slice of the xT
# strip) instead of the eleven the row-banded form needs: the 11-wide S band
# packs the contraction denser than the 6-wide R band.
#
# DMA: one load per (image, q0-strip) [128, <=2053] and one store per strip
# (outputs staged in SBUF as bf16), issued alternately on the SP and
# Activation HWDGE rings - ~2.5x fewer descriptors than per-tile DMA and two
# sequencers instead of one.  yT is written bf16 (halves store bytes); the
# host widens to fp32.  Causal masks: gpsimd affine_select on the input
# strip (keep v >= u), DVE staircase multiply on PSUM evacuation.
import sys

sys.path.insert(0, "/opt/trn_rl_repo")

import numpy as np

import concourse.bacc as bacc
import concourse.mybir as mybir
import concourse.tile as tile
from concourse.bass_utils import run_bass_kernel_spmd

N, C, H, W = 2, 16, 2048, 2048
R, S, PH, PW = 6, 11, 5, 5
NCORES = 8
IPC = (N * C) // NCORES  # images per core
QM = 118      # output cols (q) per strip: 128-partition contraction window
PN = 512      # output rows (p) per tile (one PSUM bank of fp32)
STW = 1152    # staircase width; stair2[i, t] = 1 iff t >= i + 512
F32 = mybir.dt.float32
BF16 = mybir.dt.bfloat16

_NC_CACHE = {}


def _np_bf16():
    import ml_dtypes

    return np.dtype(ml_dtypes.bfloat16)


def _strips():
    """(q0, qm) col-strips covering q in [0, H)."""
    out = []
    q0 = 0
    while q0 < H:
        out.append((q0, min(QM, H - q0)))
        q0 += QM
    return out


def _p_tiles(q0):
    """Row tiles (p0) with any causal output (p >= q0 somewhere)."""
    return [p0 for p0 in range(0, H, PN) if p0 + PN - 1 >= q0]


def _build_program(rep=1):
    """One SPMD program: conv of IPC transposed images with per-image bands.

    rep > 1 wraps the body in a hardware loop (benchmarking only)."""
    import contextlib

    nc = bacc.Bacc("TRN2", target_bir_lowering=False, debug=False,
                   num_devices=NCORES)
    xT = nc.dram_tensor("xT", [IPC, W, H], BF16, kind="ExternalInput")
    bands = nc.dram_tensor("bands", [IPC, 128, R * 128], BF16,
                           kind="ExternalInput")
    yT = nc.dram_tensor("yT", [IPC, W, H], BF16, kind="ExternalOutput")

    with tile.TileContext(nc) as tc:
        with (
            tc.tile_pool(name="const", bufs=1) as cpool,
            tc.tile_pool(name="xin", bufs=4) as xpool,
            tc.tile_pool(name="out", bufs=4) as opool,
            tc.tile_pool(name="psum", bufs=8, space="PSUM") as ppool,
            tc.For_i(0, rep, 1) if rep > 1 else contextlib.nullcontext(),
        ):
            # Staircase mask: stair2[i, t] = 1 iff t - i - 512 >= 0.
            stair2 = cpool.tile([128, STW], F32)
            nc.gpsimd.memset(stair2[:], 1.0)
            nc.gpsimd.affine_select(
                out=stair2[:], in_=stair2[:],
                compare_op=mybir.AluOpType.is_ge, fill=0.0,
                base=-512, channel_multiplier=-1,
                pattern=[[1, STW]],
            )

            ndma = [0]
            load_issuers = [nc.sync, nc.scalar]

            def dma(out, in_, nsplit=1, store=False):
                # Split along the partition dim (32-row chunks spread across
                # the SDMA engines; bigger chunks funnel onto one).  Loads
                # rotate over the two HWDGE rings; stores issue via SWDGE
                # (gpsimd) so a queued store can never head-of-line-block a
                # load the tensor engine is waiting on.
                np_ = out.shape[0]
                assert in_.shape[0] == np_
                step = (np_ + nsplit - 1) // nsplit
                for a in range(0, np_, step):
                    b = min(np_, a + step)
                    if store:
                        eng = nc.gpsimd
                    else:
                        eng = load_issuers[ndma[0] % 2]
                        ndma[0] += 1
                    eng.dma_start(out=out[a:b], in_=in_[a:b])

            bt = cpool.tile([128, IPC * R * 128], BF16)
            for i in range(IPC):
                # Band load emitted at image start: overlaps the previous
                # image's tail instead of serializing the kernel prologue.
                band_i = bt[:, i * R * 128:(i + 1) * R * 128]
                dma(band_i, bands[i], nsplit=4)
                for (q0, qm) in _strips():
                    _emit_strip(nc, tc, xpool, opool, ppool, xT, yT,
                                band_i, stair2, dma, i, q0, qm)
    nc.compile()
    return nc


def _emit_strip(nc, tc, xpool, opool, ppool, xT, yT, band_i, stair2, dma,
                i, q0, qm):
    p_tiles = _p_tiles(q0)
    ps0 = p_tiles[0]
    v0 = ps0 - PH            # strip col range [v0, H) in v = p coords
    ext = H - v0             # SBUF strip width (incl. 5-col lead halo)
    u0 = q0 - PH             # partition k = u - u0, u = input col
    uv0, uv1 = max(0, u0), min(W, u0 + 128)  # valid u rows to load

    xt = xpool.tile([128, ext], BF16, tag="xin")
    # x is causally pre-masked on the host (xT[u, v] = 0 for v < u), so no
    # in-kernel selects: just zero the regions the DMA does not write.
    # Cols v < u0 are entirely below every u in this strip's window (all
    # masked zeros): memset instead of loading.
    lv0 = max(0, v0, u0)
    if u0 + 128 > W:
        # last strip: partitions k >= W-u0 are u >= W (right w-padding).
        # Compute-engine partition bases must be 32-aligned, so zero the
        # whole tile first; the load below overwrites the valid rows (WAW).
        nc.vector.memset(xt[:, :], 0.0)
    dma(xt[uv0 - u0:uv1 - u0, lv0 - v0:], xT[i, uv0:uv1, lv0:H], nsplit=4)
    if lv0 > v0:
        # covers both the below-diagonal clip and the v<0 top padding
        nc.vector.memset(xt[:, :lv0 - v0], 0.0)
    if u0 < 0:
        # q0 == 0: partitions k < 5 are u < 0 (left w-padding): zero them.
        nc.vector.memset(xt[:PH, :], 0.0)

    ys = opool.tile([128, ext], BF16, tag="out")
    # r-outer / p-inner: consecutive matmuls share the stationary band B_r;
    # the explicit ldweights lets codegen elide the per-matmul weight load.
    pts = {}
    for p0 in p_tiles:
        pt = ppool.tile([128, PN], F32, tag="psum")
        pts[p0] = pt
    for r in range(R):
        band_r = band_i[:, r * 128:(r + 1) * 128]
        for p0 in p_tiles:
            # moving operand: xT strip cols v = p0-5+r .. +PN.  Cols left
            # of the causal diagonal (n < n0) are masked on evacuation:
            # skip streaming them.
            n0 = max(0, q0 - p0)
            j0 = p0 - PH + r - v0
            nc.tensor.matmul(
                pts[p0][:, n0:PN],
                lhsT=band_r,
                rhs=xt[:, j0 + n0:j0 + PN],
                start=(r == 0), stop=(r == R - 1),
            )
    for p0 in p_tiles:
        pt = pts[p0]
        # Evacuate cols p >= q0 only (left of that is all-masked).
        n0 = max(0, q0 - p0)
        d = p0 - q0
        crossing = p0 + n0 < q0 + qm - 1
        if crossing:
            nc.vector.tensor_mul(
                ys[:qm, p0 + n0 - v0:p0 + PN - v0],
                pt[:qm, n0:PN],
                stair2[:qm, 512 + d + n0:512 + d + PN],
            )
        else:
            nc.vector.tensor_copy(
                ys[:qm, p0 + n0 - v0:p0 + PN - v0], pt[:qm, n0:PN],
            )
    # One store per strip: cols p in [q0, H).
    dma(yT[i, q0:q0 + qm, q0:H], ys[:qm, q0 - v0:], nsplit=4, store=True)


def _build_bands(weight):
    """Host-side: per-image column-banded Toeplitz weights.
    bands[img, k, r*128 + m] = w[c(img), r, k-m] for k-m in [0, S)."""
    nimg = N * C
    bands = np.zeros((nimg, 128, R * 128), np.float32)
    m = np.arange(128)
    for r in range(R):
        for s in range(S):
            valid = m + s < 128
            mv = m[valid]
            for img in range(nimg):
                c = img % C
                bands[img, mv + s, r * 128 + mv] = weight[c, r, s]
    return bands.astype(_np_bf16())


def _prep_xT(x):
    """Causally masked (h >= w), transposed, bf16 images [N*C, W, H]."""
    mask = np.tril(np.ones((H, W), np.float32))
    xm = x.reshape(N * C, H, W) * mask
    return np.ascontiguousarray(xm.swapaxes(1, 2)).astype(
        _np_bf16(), copy=False)


def kernel(x, weight):
    x = np.asarray(x, dtype=np.float32)
    weight = np.asarray(weight, dtype=np.float32)
    assert x.shape == (N, C, H, W) and weight.shape == (C, R, S)

    if "nc" not in _NC_CACHE:
        _NC_CACHE["nc"] = _build_program()
    nc = _NC_CACHE["nc"]

    xT_imgs = _prep_xT(x)
    bands = _build_bands(weight)
    in_maps = [
        {
            "xT": xT_imgs[k * IPC:(k + 1) * IPC],
            "bands": bands[k * IPC:(k + 1) * IPC],
        }
        for k in range(NCORES)
    ]
    res = run_bass_kernel_spmd(nc, in_maps, list(range(NCORES)))
    yT = np.concatenate([res.results[k]["yT"] for k in range(NCORES)], axis=0)
    y = yT.astype(np.float32).swapaxes(1, 2)
    return np.ascontiguousarray(y).reshape(N, C, H, W)
